# revision 1
# baseline (speedup 1.0000x reference)
"""Trainium2 Bass kernel for DigitConvolutionalModel.

Computes: out = relu(conv2d_valid(x.reshape(B,28,28), w3x3)).reshape(B,676) @ W + b

Strategy (pure data parallel over 8 NeuronCores, 8192 images/core):
  - Host: transpose x to pixel-major xT[784, B] so the conv contraction dim
    (pixels) lands on SBUF partitions; build two small constant matrices
    CA[112,104], CB[56,104] that express the 3x3 valid conv as matmuls over
    4-image-row input chunks; chunk W by 104 output features.
  - Device per core: stream 512-image blocks. Each block:
      1 DMA load of xT tile [112 part, 7 chunks, 512 batch]  (1.6 MB)
      conv: 7 output groups; groups 0..5 = 2 accumulating PE matmuls
            (CA against chunk t, CB against first 2 rows of chunk t+1),
            group 6 = 1 matmul. PSUM [104, 512].
      ReLU PSUM->SBUF on ACT/DVE -> feat tiles (the conv features).
      fc: 7 accumulating PE matmuls (W chunk [104,10] as stationary,
          feat [104,512] moving) -> PSUM [10, 512]; bias-add via ACT copy.
      1 DMA store of outT [10, 512].
  - Host: gather per-core outT [10, 8192] and transpose into out[B, 10].

Matmul dtype: float32r (fp32 storage, 1 row/cycle on the PE for N>=256)
by default; switchable to bf16 / exact f32 via KERNEL_PRECISION env var.
"""

import os

import numpy as np

import concourse.bass as bass
import concourse.mybir as mybir
import concourse.tile as tile
from concourse import bacc
from concourse.bass import ts
from concourse.bass_utils import run_bass_kernel_spmd

# Problem geometry (fixed by the task spec)
B_FULL = 65536
IMG = 28
KW = 3
OH = IMG - KW + 1          # 26
NPIX = IMG * IMG           # 784
NFEAT = OH * OH            # 676
NOUT = 10

N_CORES = 8
B_CORE = B_FULL // N_CORES  # 8192
NB = 512                    # images per block
N_BLOCKS = B_CORE // NB     # 16

# Conv grouping: output rows in groups of 4 -> 104 features per group
G_ROWS = 4
G_FEAT = G_ROWS * OH        # 104
N_GROUPS = 7                # 6 full groups of 4 out-rows + 1 group of 2 (52 feats)
CHUNK_ROWS = 4              # input rows per SBUF partition chunk
CHUNK_PIX = CHUNK_ROWS * IMG  # 112
N_CHUNKS = NPIX // CHUNK_PIX  # 7

PRECISION = os.environ.get("KERNEL_PRECISION", "f32r")  # f32r | bf16 | f32


def _feat_count(t: int) -> int:
    # group t covers output rows [4t, min(4t+4, 26)) -> features [104t, ...)
    return min(G_FEAT, NFEAT - G_FEAT * t)


def build_selector() -> np.ndarray:
    """S[64g + o, o] = 1: sums the 2 col-tiled partial groups of pw."""
    S = np.zeros((128, NOUT), np.float32)
    for g in range(2):
        for o in range(NOUT):
            S[64 * g + o, o] = 1.0
    return S


def build_conv_mats(conv_w: np.ndarray):
    """CA[(ri,c),(ol,oj)] and CB[(ri2,c),(ol,oj)] express the 3x3 valid conv.

    For output group rows ol in [0,4): feature (ol, oj) gets contributions
      from chunk t rows ri (absolute 4t+ri):   w[ri-ol, c-oj]
      from chunk t+1 rows ri2 (absolute 4t+4+ri2): w[4+ri2-ol, c-oj]
    """
    w = np.asarray(conv_w, np.float32)
    CA = np.zeros((CHUNK_PIX, G_FEAT), np.float32)
    CB = np.zeros((2 * IMG, G_FEAT), np.float32)
    for ol in range(G_ROWS):
        for oj in range(OH):
            m = ol * OH + oj
            for di in range(KW):
                for dj in range(KW):
                    r = ol + di          # row within the 4-row window
                    c = oj + dj
                    if r < CHUNK_ROWS:
                        CA[r * IMG + c, m] = w[di, dj]
                    else:
                        CB[(r - CHUNK_ROWS) * IMG + c, m] = w[di, dj]
    return CA, CB


def build_program():
    f32 = mybir.dt.float32
    if PRECISION == "bf16":
        st_dt = mybir.dt.bfloat16
    elif PRECISION == "f32":
        st_dt = f32
    else:
        # float32r end-to-end: the BIR verifier requires every producer of a
        # fp32r-matmul operand to emit fp32r (DMA loads, ReLU copies).
        st_dt = mybir.dt.float32r

    # Bacc (not raw Bass): its compile pipeline splits multi-sem waits into
    # event semaphores — TRN2 instructions carry at most ONE sync wait.
    nc = bacc.Bacc()
    xT = nc.declare_dram_parameter("xT", [NPIX, B_CORE], st_dt, isOutput=False)
    ca_d = nc.declare_dram_parameter("CA", [CHUNK_PIX, G_FEAT], st_dt, isOutput=False)
    cb_d = nc.declare_dram_parameter("CB", [2 * IMG, G_FEAT], st_dt, isOutput=False)
    wp_d = nc.declare_dram_parameter("Wp", [G_FEAT, N_GROUPS, 32], st_dt, isOutput=False)
    sel_d = nc.declare_dram_parameter("Sel", [128, NOUT], st_dt, isOutput=False)
    bias_d = nc.declare_dram_parameter("bias", [NOUT, 1], f32, isOutput=False)
    outT = nc.declare_dram_parameter("outT", [NOUT, B_CORE], f32, isOutput=True)

    def mm(ap):
        return ap

    Relu = mybir.ActivationFunctionType.Relu
    Ident = mybir.ActivationFunctionType.Identity

    with tile.TileContext(nc) as tc:
        with (
            tc.tile_pool(name="const", bufs=1) as const,
            tc.tile_pool(name="x", bufs=4) as xpool,
            tc.tile_pool(name="feat", bufs=14) as fpool,
            tc.tile_pool(name="osb", bufs=3) as opool,
            tc.tile_pool(name="psc", bufs=7, space="PSUM") as psc,
            tc.tile_pool(name="pso", bufs=1, space="PSUM") as pso,
        ):
            # Constants go on the ACT HWDGE ring so they land immediately,
            # in parallel with the x-block loads on the SP ring.
            ca_sb = const.tile([CHUNK_PIX, G_FEAT], st_dt)
            nc.scalar.dma_start(out=ca_sb[:, :], in_=ca_d[:, :])
            cb_sb = const.tile([2 * IMG, G_FEAT], st_dt)
            nc.scalar.dma_start(out=cb_sb[:, :], in_=cb_d[:, :])
            wp_sb = const.tile([G_FEAT, N_GROUPS, 32], st_dt)
            nc.scalar.dma_start(out=wp_sb[:, :, :], in_=wp_d[:, :, :])
            bias_sb = const.tile([NOUT, 1], f32)
            nc.scalar.dma_start(out=bias_sb[:, :], in_=bias_d[:, :])

            # PE warm-up: ~5us of gapless matmuls during the first x DMA so
            # the HAM clock gate opens (1.2 -> 2.4 GHz) before real work.
            # CA serves as both operands (N=104); output is never read.
            warm_ps = psc.tile([G_FEAT, NB], mybir.dt.float32, tag="convps")
            for _ in range(32):
                nc.tensor.matmul(
                    warm_ps[:, :G_FEAT], mm(ca_sb[:, :]), mm(ca_sb[:, :]),
                    start=True, stop=True,
                )

            def emit_block(xt, j):
                # A-phase: all 7 groups against CA (one stationary weight
                # set, gapless PE stream keeps the HAM clock warm).
                pss = []
                for t in range(N_GROUPS):
                    nf = _feat_count(t)
                    ps = psc.tile([nf, NB], mybir.dt.float32, tag="convps")
                    nc.tensor.matmul(
                        ps[:, :], mm(ca_sb[:, :nf]), mm(xt[:, t, :]),
                        start=True, stop=(t == N_GROUPS - 1),
                    )
                    pss.append(ps)
                # B-phase: close groups 0..5 against CB.
                for t in range(N_GROUPS - 1):
                    nc.tensor.matmul(
                        pss[t][:, :], mm(cb_sb[:, :]),
                        mm(xt[: 2 * IMG, t + 1, :]),
                        start=False, stop=True,
                    )
                # ReLU PSUM->SBUF (4 on ACT, 3 on DVE).
                feats = []
                for t in range(N_GROUPS):
                    nf = _feat_count(t)
                    ft = fpool.tile([nf, NB], st_dt, tag="feat")
                    if t % 2 == 0:
                        nc.scalar.activation(ft[:, :], pss[t][:, :], Relu)
                    else:
                        nc.vector.tensor_scalar_max(ft[:, :], pss[t][:, :], 0.0)
                    feats.append(ft)

                # W-phase: 7 accumulating matmuls (tiny 10-col LDWs).
                ops = pso.tile([NOUT, NB], mybir.dt.float32, tag="outps")
                for t in range(N_GROUPS):
                    nf = _feat_count(t)
                    nc.tensor.matmul(
                        ops[:, :], mm(wp_sb[:nf, t, :NOUT]), mm(feats[t][:nf, :]),
                        start=(t == 0), stop=(t == N_GROUPS - 1),
                    )
                osb = opool.tile([NOUT, NB], f32, tag="osb")
                nc.vector.tensor_scalar(
                    osb[:, :], ops[:, :], bias_sb[:, :], None,
                    op0=mybir.AluOpType.add,
                )
                nc.sync.dma_start(out=outT[:, ts(j, NB)], in_=osb[:, :])

            for j in range(N_BLOCKS):
                xt = xpool.tile([CHUNK_PIX, N_CHUNKS, NB], st_dt, tag="x")
                srcap = xT[:, ts(j, NB)].rearrange(
                    "(c p) b -> p c b", p=CHUNK_PIX
                )
                nc.sync.dma_start(out=xt[:, :, :], in_=srcap)
                emit_block(xt, j)

    nc.finalize()  # runs Bacc.compile(): wait-splitting + register allocation
    return nc


def _np_dt():
    if PRECISION == "bf16":
        import ml_dtypes

        return ml_dtypes.bfloat16
    return np.float32


def prepare_inputs(x, conv_w, W, b):
    dt = _np_dt()
    xT = np.ascontiguousarray(np.asarray(x, np.float32).T.astype(dt))
    CA, CB = build_conv_mats(conv_w)
    Wf = np.asarray(W, np.float32)
    Wp = np.zeros((G_FEAT, N_GROUPS, 32), np.float32)
    for t in range(N_GROUPS):
        nf = _feat_count(t)
        Wp[:nf, t, :NOUT] = Wf[G_FEAT * t : G_FEAT * t + nf, :]
    bias = np.asarray(b, np.float32).reshape(NOUT, 1)
    Sel = build_selector()
    CA, CB, Wp, Sel = CA.astype(dt), CB.astype(dt), Wp.astype(dt), Sel.astype(dt)
    in_maps = []
    for c in range(N_CORES):
        in_maps.append(
            {
                "xT": np.ascontiguousarray(xT[:, c * B_CORE : (c + 1) * B_CORE]),
                "CA": CA,
                "CB": CB,
                "Wp": Wp,
                "Sel": Sel,
                "bias": bias,
            }
        )
    return in_maps


def _enable_ldw_opt():
    """Let walrus dedup/overlap repeated LDWEIGHTS (safe for the f32r
    self-loading matmul form; incompatible with bf16's standalone LDW)."""
    import concourse.bass_utils as bu

    if getattr(bu, "_ldw_opt_patched", False):
        return
    orig = bu.run_command

    def patched(argv, **kw):
        argv = [
            "--enable-ldw-opt=true" if a == "--enable-ldw-opt=false" else a
            for a in argv
        ]
        return orig(argv, **kw)

    bu.run_command = patched
    bu._ldw_opt_patched = True


def run(x, conv_w, W, b, trace=False, **spmd_kwargs):
    if PRECISION == "f32r":
        _enable_ldw_opt()
    in_maps = prepare_inputs(x, conv_w, W, b)
    nc = build_program()
    res = run_bass_kernel_spmd(
        nc, in_maps, list(range(N_CORES)), trace=trace, **spmd_kwargs
    )
    out = np.empty((B_FULL, NOUT), np.float32)
    for c in range(N_CORES):
        out[c * B_CORE : (c + 1) * B_CORE, :] = res.results[c]["outT"].T
    return out, res


def kernel(x, conv_w, W, b):
    out, _ = run(x, conv_w, W, b, trace=False)
    return out



# revision 5
# speedup vs baseline: 1.4142x; 1.4142x over previous
"""Trainium2 Bass kernel for DigitConvolutionalModel.

Computes: out = relu(conv2d_valid(x.reshape(B,28,28), w3x3)).reshape(B,676) @ W + b

Strategy (pure data parallel over 8 NeuronCores, 8192 images/core):
  - Host: swizzle x per core to xTs[112, 16 blocks, 7 chunks, 512 batch] bf16 so
    each block's DMA reads one 7KB-contiguous segment per partition; build two
    small constant matrices CA[112,104], CB[56,104] that express the 3x3 valid
    conv as matmuls over 4-image-row input chunks; chunk W by 104 features.
  - Device per core: stream 512-image blocks, grouped in megas of 4 blocks.
      conv per block: 7 output groups; groups 0..5 = 2 accumulating PE matmuls
        (CA against chunk t, CB against first 2 rows of chunk t+1), group 6 = 1
        matmul. PSUM [104, 512] x7. ReLU PSUM->SBUF on ACT/DVE -> bf16 feats.
      fc per mega: 7x4 accumulating PE matmuls, stationary W chunk reused
        across the 4 blocks (amortizes LDWEIGHTS); all 4 output chains live in
        one shared PSUM bank [40, 512] at partition offsets 0/10/20/30.
      bias-add via DVE tensor_scalar; 1 DMA store of outT [10, 512] per block.
  - Host: gather per-core outT [10, 8192] and transpose into out[B, 10].

Matmul dtype: bfloat16 (fp32 PSUM accumulation). Total HBM read per core is
12.9 MB (bf16 x) vs 25.7 MB for f32 — the kernel targets the memory roofline.
"""

import os

import numpy as np

import concourse.bass as bass
import concourse.mybir as mybir
import concourse.tile as tile
from concourse import bacc
from concourse.bass import ts
from concourse.bass_utils import run_bass_kernel_spmd

# Problem geometry (fixed by the task spec)
B_FULL = 65536
IMG = 28
KW = 3
OH = IMG - KW + 1          # 26
NPIX = IMG * IMG           # 784
NFEAT = OH * OH            # 676
NOUT = 10

N_CORES = 8
B_CORE = B_FULL // N_CORES  # 8192
NB = 512                    # images per block
N_BLOCKS = B_CORE // NB     # 16
MEGA = 2                    # blocks per fc mega-phase
N_MEGA = N_BLOCKS // MEGA   # 8
OPS_BASE = 64               # partition stride of fc chains in the shared bank
                            # (matmul PSUM out base must be 0, 32, or 64)

# Conv grouping: output rows in groups of 4 -> 104 features per group
G_ROWS = 4
G_FEAT = G_ROWS * OH        # 104
N_GROUPS = 7                # 6 full groups of 4 out-rows + 1 group of 2 (52 feats)
CHUNK_ROWS = 4              # input rows per SBUF partition chunk
CHUNK_PIX = CHUNK_ROWS * IMG  # 112
N_CHUNKS = NPIX // CHUNK_PIX  # 7

PRECISION = os.environ.get("KERNEL_PRECISION", "bf16")  # bf16 | f32r


def _feat_count(t: int) -> int:
    # group t covers output rows [4t, min(4t+4, 26)) -> features [104t, ...)
    return min(G_FEAT, NFEAT - G_FEAT * t)


def build_conv_mats(conv_w: np.ndarray):
    """CA[(ri,c),(ol,oj)] and CB[(ri2,c),(ol,oj)] express the 3x3 valid conv.

    For output group rows ol in [0,4): feature (ol, oj) gets contributions
      from chunk t rows ri (absolute 4t+ri):   w[ri-ol, c-oj]
      from chunk t+1 rows ri2 (absolute 4t+4+ri2): w[4+ri2-ol, c-oj]
    """
    w = np.asarray(conv_w, np.float32)
    CA = np.zeros((CHUNK_PIX, G_FEAT), np.float32)
    CB = np.zeros((2 * IMG, G_FEAT), np.float32)
    for ol in range(G_ROWS):
        for oj in range(OH):
            m = ol * OH + oj
            for di in range(KW):
                for dj in range(KW):
                    r = ol + di          # row within the 4-row window
                    c = oj + dj
                    if r < CHUNK_ROWS:
                        CA[r * IMG + c, m] = w[di, dj]
                    else:
                        CB[(r - CHUNK_ROWS) * IMG + c, m] = w[di, dj]
    return CA, CB


def build_program():
    f32 = mybir.dt.float32
    st_dt = mybir.dt.float32r if PRECISION == "f32r" else mybir.dt.bfloat16

    # Bacc (not raw Bass): its compile pipeline splits multi-sem waits into
    # event semaphores — TRN2 instructions carry at most ONE sync wait.
    nc = bacc.Bacc()
    xTs = nc.declare_dram_parameter(
        "xTs", [CHUNK_PIX, N_BLOCKS, N_CHUNKS, NB], st_dt, isOutput=False
    )
    ca_d = nc.declare_dram_parameter("CA", [CHUNK_PIX, G_FEAT], st_dt, isOutput=False)
    cb_d = nc.declare_dram_parameter("CB", [2 * IMG, G_FEAT], st_dt, isOutput=False)
    wp_d = nc.declare_dram_parameter("Wp", [G_FEAT, N_GROUPS, 32], st_dt, isOutput=False)
    bias_d = nc.declare_dram_parameter("bias", [NOUT, 1], f32, isOutput=False)
    outT = nc.declare_dram_parameter("outT", [NOUT, B_CORE], f32, isOutput=True)

    Relu = mybir.ActivationFunctionType.Relu

    with tile.TileContext(nc) as tc:
        with (
            tc.tile_pool(name="const", bufs=1) as const,
            tc.tile_pool(name="x", bufs=6) as xpool,
            tc.tile_pool(name="feat", bufs=2 * MEGA * N_GROUPS + 4) as fpool,
            tc.tile_pool(name="osb", bufs=2 * MEGA) as opool,
            tc.tile_pool(name="psc", bufs=7, space="PSUM") as psc,
            tc.tile_pool(name="pso", bufs=1, space="PSUM") as pso,
        ):
            # Constants go on the ACT HWDGE ring so they land immediately,
            # in parallel with the x-block loads on the SP ring.
            ca_sb = const.tile([CHUNK_PIX, G_FEAT], st_dt)
            nc.scalar.dma_start(out=ca_sb[:, :], in_=ca_d[:, :])
            cb_sb = const.tile([2 * IMG, G_FEAT], st_dt)
            nc.scalar.dma_start(out=cb_sb[:, :], in_=cb_d[:, :])
            wp_sb = const.tile([G_FEAT, N_GROUPS, 32], st_dt)
            nc.scalar.dma_start(out=wp_sb[:, :, :], in_=wp_d[:, :, :])
            bias_sb = const.tile([NOUT, 1], f32)
            nc.scalar.dma_start(out=bias_sb[:, :], in_=bias_d[:, :])

            # PE warm-up: gapless matmuls during the first x DMA so the HAM
            # clock gate opens (1.2 -> 2.4 GHz) before real work. CA serves as
            # both operands (N=104); output is never read.
            warm_ps = psc.tile([G_FEAT, NB], mybir.dt.float32, tag="convps")
            for _ in range(32):
                nc.tensor.matmul(
                    warm_ps[:, :G_FEAT], ca_sb[:, :], ca_sb[:, :],
                    start=True, stop=True,
                )

            def emit_conv(xt, feats_out):
                # A-phase: all 7 groups against CA (one stationary weight
                # set, gapless PE stream keeps the HAM clock warm).
                pss = []
                for t in range(N_GROUPS):
                    nf = _feat_count(t)
                    ps = psc.tile([nf, NB], mybir.dt.float32, tag="convps")
                    nc.tensor.matmul(
                        ps[:, :], ca_sb[:, :nf], xt[:, t, :],
                        start=True, stop=(t == N_GROUPS - 1),
                    )
                    pss.append(ps)
                # B-phase: close groups 0..5 against CB.
                for t in range(N_GROUPS - 1):
                    nc.tensor.matmul(
                        pss[t][:, :], cb_sb[:, :],
                        xt[: 2 * IMG, t + 1, :],
                        start=False, stop=True,
                    )
                # ReLU PSUM->SBUF (4 on ACT, 3 on DVE).
                for t in range(N_GROUPS):
                    nf = _feat_count(t)
                    ft = fpool.tile([nf, NB], st_dt, tag="feat")
                    if t % 2 == 0:
                        nc.scalar.activation(ft[:, :], pss[t][:, :], Relu)
                    else:
                        nc.vector.tensor_scalar_max(ft[:, :], pss[t][:, :], 0.0)
                    feats_out.append(ft)

            for m in range(N_MEGA):
                mega_feats = []
                for jj in range(MEGA):
                    j = m * MEGA + jj
                    xt = xpool.tile([CHUNK_PIX, N_CHUNKS, NB], st_dt, tag="x")
                    nc.sync.dma_start(out=xt[:, :, :], in_=xTs[:, j, :, :])
                    feats = []
                    emit_conv(xt, feats)
                    mega_feats.append(feats)

                # fc phase for the mega: stationary W chunk t reused across the
                # blocks; the accumulation chains share one PSUM bank at
                # partition bases 0/64 (matmul out base must be 0, 32, or 64).
                ops = pso.tile(
                    [(MEGA - 1) * OPS_BASE + NOUT, NB],
                    mybir.dt.float32, tag="outps",
                )
                for t in range(N_GROUPS):
                    nf = _feat_count(t)
                    for jj in range(MEGA):
                        nc.tensor.matmul(
                            ops[jj * OPS_BASE : jj * OPS_BASE + NOUT, :],
                            wp_sb[:nf, t, :NOUT],
                            mega_feats[jj][t][:nf, :],
                            start=(t == 0), stop=(t == N_GROUPS - 1),
                        )
                for jj in range(MEGA):
                    j = m * MEGA + jj
                    osb = opool.tile([NOUT, NB], f32, tag="osb")
                    nc.vector.tensor_scalar(
                        osb[:, :], ops[jj * OPS_BASE : jj * OPS_BASE + NOUT, :],
                        bias_sb[:, :], None,
                        op0=mybir.AluOpType.add,
                    )
                    nc.scalar.dma_start(out=outT[:, ts(j, NB)], in_=osb[:, :])

    nc.finalize()  # runs Bacc.compile(): wait-splitting + register allocation
    return nc


def _np_dt():
    if PRECISION == "f32r":
        return np.float32
    import ml_dtypes

    return ml_dtypes.bfloat16


def prepare_inputs(x, conv_w, W, b):
    dt = _np_dt()
    xf = np.asarray(x, np.float32)
    CA, CB = build_conv_mats(conv_w)
    Wf = np.asarray(W, np.float32)
    Wp = np.zeros((G_FEAT, N_GROUPS, 32), np.float32)
    for t in range(N_GROUPS):
        nf = _feat_count(t)
        Wp[:nf, t, :NOUT] = Wf[G_FEAT * t : G_FEAT * t + nf, :]
    bias = np.asarray(b, np.float32).reshape(NOUT, 1)
    CA, CB, Wp = CA.astype(dt), CB.astype(dt), Wp.astype(dt)
    in_maps = []
    for c in range(N_CORES):
        xc = xf[c * B_CORE : (c + 1) * B_CORE]  # [8192, 784]
        # [p, block, chunk, batch]: per (p, block) the [chunk, batch] plane is
        # contiguous -> 7KB DMA lines.
        xts = (
            xc.reshape(N_BLOCKS, NB, N_CHUNKS, CHUNK_PIX)
            .transpose(3, 0, 2, 1)
            .astype(dt)
        )
        in_maps.append(
            {
                "xTs": np.ascontiguousarray(xts),
                "CA": CA,
                "CB": CB,
                "Wp": Wp,
                "bias": bias,
            }
        )
    return in_maps


def _enable_ldw_opt():
    """Let walrus dedup/overlap repeated LDWEIGHTS (safe for the f32r
    self-loading matmul form; incompatible with bf16's standalone LDW)."""
    import concourse.bass_utils as bu

    if getattr(bu, "_ldw_opt_patched", False):
        return
    orig = bu.run_command

    def patched(argv, **kw):
        argv = [
            "--enable-ldw-opt=true" if a == "--enable-ldw-opt=false" else a
            for a in argv
        ]
        return orig(argv, **kw)

    bu.run_command = patched
    bu._ldw_opt_patched = True


def run(x, conv_w, W, b, trace=False, **spmd_kwargs):
    if PRECISION == "f32r":
        _enable_ldw_opt()
    in_maps = prepare_inputs(x, conv_w, W, b)
    nc = build_program()
    res = run_bass_kernel_spmd(
        nc, in_maps, list(range(N_CORES)), trace=trace, **spmd_kwargs
    )
    out = np.empty((B_FULL, NOUT), np.float32)
    for c in range(N_CORES):
        out[c * B_CORE : (c + 1) * B_CORE, :] = res.results[c]["outT"].T
    return out, res


def kernel(x, conv_w, W, b):
    out, _ = run(x, conv_w, W, b, trace=False)
    return out
